# revision 1
# baseline (speedup 1.0000x reference)
"""Trainium2 Bass kernel v2 for a GPT decoder block (d=768, H=12, S=4096, FFN=3072).

Sharding: 8-way SPMD over query-row blocks, cyclic (core c owns blocks
{c, c+8, c+16, c+24}).  Every core builds K/V for the full sequence.

Design (vs the v1 baseline):
- LayerNorm folded into the projections: host folds gamma/beta into weights
  (W~ = diag(gamma) W, bias += beta^T W); the device computes per-row
  a = rstd/2, b = -mu*rstd/2; normalization is one fused DVE multiply
  (xa8 = x^T * a_bcast -> fp8) plus a rank-1 correction row in each matmul.
- fp8(e4m3) DoubleRow matmuls (2x PE) for Q/K/V/FFN/scores.  Weights
  prescaled by 32, activations by 1/2, so PSUM lands in drain units; biases
  enter as rank-1 rows against a resident [b; ones] operand, making drains
  pure casts.  FFN weights use a hi+lo fp8 split (~11-bit effective).
- rstd via exp(-0.5 ln(var+eps)): Ln/Exp share an activation table set with
  the attention Exp, so there are no act-table reloads in the main pipeline.
- Scores are 32-partition DoubleRow fp8; AV uses fp8 probs + DoubleRow for
  fully-live groups, bf16 probs for the diagonal group (the only masked one).
- DMA-engine (xbar) transposes for all layout changes; per-2-superblock
  stats/K/V/attention pipelining; Wo + LN2 pipelined per query block.
"""

import os
import sys
from contextlib import ExitStack

import numpy as np
import ml_dtypes

for _p in ("/opt/trn_rl_repo", "/opt/pypackages"):
    if os.path.isdir(_p) and _p not in sys.path:
        sys.path.append(_p)

import concourse.bacc as bacc
import concourse.tile as tile
from concourse import mybir
from concourse.bass_utils import run_bass_kernel_spmd

P = 128
D = 768
DT = 6
H = 12
HD = 64
S = 4096
NB = 32
SB = 8
HID = 3072
FT = 24
QB = 4
NCORES = 8
EPS = 1e-5
SCALE = 0.125
XS = 0.5                 # activation prescale (a carries XS*rstd)
WS = 32.0                # weight fp8 prescale
KS = XS * WS             # = 16: scale of k8/q8/v8
ESC = SCALE / (KS * KS)  # exp() scale on score psum
HD1 = HD + 1

F32 = mybir.dt.float32
BF16 = mybir.dt.bfloat16
FP8 = mybir.dt.float8e4
AF = mybir.ActivationFunctionType
DRM = mybir.MatmulPerfMode.DoubleRow
MUL = mybir.AluOpType.mult
ADD = mybir.AluOpType.add

bfdt = ml_dtypes.bfloat16
f8dt = ml_dtypes.float8_e4m3

_CACHE = {}


def _build():
    nc = bacc.Bacc("TRN2", target_bir_lowering=False, debug=False,
                   num_devices=NCORES)

    # ---- DRAM I/O ----
    xrow_d = nc.dram_tensor("xrow", [S, D], BF16, kind="ExternalInput").ap()
    xT_d = nc.dram_tensor("xT", [D, S], BF16, kind="ExternalInput").ap()
    xq_d = nc.dram_tensor("xq", [QB * P, D], BF16, kind="ExternalInput").ap()
    xqT_d = nc.dram_tensor("xqT", [D, QB * P], BF16, kind="ExternalInput").ap()
    wk8_d = nc.dram_tensor("wk8", [D, D], FP8, kind="ExternalInput").ap()
    wq8_d = nc.dram_tensor("wq8", [D, D], FP8, kind="ExternalInput").ap()
    wv8_d = nc.dram_tensor("wv8", [D, D], FP8, kind="ExternalInput").ap()
    wo_d = nc.dram_tensor("wo", [D, D], BF16, kind="ExternalInput").ap()
    w18_d = nc.dram_tensor("w18", [D, HID], FP8, kind="ExternalInput").ap()
    w28_d = nc.dram_tensor("w28", [HID, D], FP8, kind="ExternalInput").ap()
    w18lo_d = nc.dram_tensor("w18lo", [D, HID], FP8, kind="ExternalInput").ap()
    w28lo_d = nc.dram_tensor("w28lo", [HID, D], FP8, kind="ExternalInput").ap()
    csk_d = nc.dram_tensor("csk", [32, 2, D], FP8, kind="ExternalInput").ap()
    csq_d = nc.dram_tensor("csq", [32, 2, D], FP8, kind="ExternalInput").ap()
    csv_d = nc.dram_tensor("csv", [32, 2, D], FP8, kind="ExternalInput").ap()
    csw1_d = nc.dram_tensor("csw1", [32, 2, HID], FP8, kind="ExternalInput").ap()
    csw2_d = nc.dram_tensor("csw2", [32, 2, D], FP8, kind="ExternalInput").ap()
    cso_d = nc.dram_tensor("cso", [32, 2, D], FP8, kind="ExternalInput").ap()
    killd_d = nc.dram_tensor("killd", [P, QB * 8 * P], BF16,
                             kind="ExternalInput").ap()
    bones_d = nc.dram_tensor("bones", [32, 2, S], FP8,
                             kind="ExternalInput").ap()
    out_d = nc.dram_tensor("out", [QB * P, D], BF16, kind="ExternalOutput").ap()
    scra_d = nc.dram_tensor("scra", [1, S], BF16, kind="Internal").ap()
    scrb_d = nc.dram_tensor("scrb", [1, S], BF16, kind="Internal").ap()
    scraq_d = nc.dram_tensor("scraq", [1, QB * P], BF16, kind="Internal").ap()
    scrbq_d = nc.dram_tensor("scrbq", [1, QB * P], BF16, kind="Internal").ap()
    scra2_d = nc.dram_tensor("scra2", [1, QB * P], BF16, kind="Internal").ap()
    scrb2_d = nc.dram_tensor("scrb2", [1, QB * P], BF16, kind="Internal").ap()

    with tile.TileContext(nc) as tc, ExitStack() as root:
        singles = root.enter_context(tc.tile_pool(name="singles", bufs=1))

        eps_t = singles.tile([P, 1], F32)
        nc.vector.memset(eps_t, EPS)
        lnxs_t = singles.tile([P, 1], F32)
        nc.vector.memset(lnxs_t, float(np.log(XS)))

        xqT_sb = singles.tile([P, DT, QB * P], BF16)
        nc.sync.dma_start(out=xqT_sb,
                          in_=xqT_d.rearrange("(u p) s -> p u s", p=P))
        attn_sb = singles.tile([P, QB, D], BF16)
        x1T = singles.tile([P, DT, QB * P], BF16)
        q8 = singles.tile([P, 2, 3, QB * P], FP8)
        k8 = singles.tile([P, 2, 3, S], FP8)
        v8 = singles.tile([P, NB, H, HD1], FP8)
        nc.vector.memset(v8[:, :, :, HD:HD1], 1.0)
        bq_pad = singles.tile([32, 2, QB * P], FP8)
        nc.sync.dma_start(out=bq_pad, in_=bones_d[:, :, 0:QB * P])

        def bn_block(pool, src_ap, mv_dst):
            stats = pool.tile([P, 3, nc.vector.BN_STATS_DIM], F32, tag="st")
            xg = src_ap.rearrange("p (g f) -> p g f", g=3)
            for g in range(3):
                nc.vector.bn_stats(out=stats[:, g, :], in_=xg[:, g, :])
            nc.vector.bn_aggr(out=mv_dst, in_=stats)

        def rstd_batch(pool, mv_cols, a_cols, b_cols):
            """[P, n, 2] (mean,var) -> a [P,n] = XS*rstd, b = -XS*mu*rstd."""
            n = mv_cols.shape[1]
            lnz = pool.tile([P, n], F32, tag="lnz")
            nc.scalar.activation(out=lnz, in_=mv_cols[:, :, 1], func=AF.Ln,
                                 bias=eps_t, scale=1.0)
            a_f = pool.tile([P, n], F32, tag="af")
            nc.scalar.activation(out=a_f, in_=lnz, func=AF.Exp,
                                 bias=lnxs_t, scale=-0.5)
            nc.vector.tensor_copy(out=a_cols, in_=a_f)
            nc.vector.scalar_tensor_tensor(out=b_cols, in0=mv_cols[:, :, 0],
                                           scalar=-1.0, in1=a_f,
                                           op0=MUL, op1=MUL)

        def bounce(pool, a_blk, b_blk, scr_a, scr_b, r0, r1, d0=0):
            nr = r1 - r0
            aT = pool.tile([P, 1, P], BF16, tag="aT")
            nc.sync.dma_start_transpose(out=aT, in_=a_blk)
            nc.sync.dma_start(
                out=scr_a.rearrange("o (b p) -> (o b) p", p=P)[d0:d0 + nr, :],
                in_=aT[r0:r1, 0, :])
            bT = pool.tile([P, 1, P], BF16, tag="bT")
            nc.sync.dma_start_transpose(out=bT, in_=b_blk)
            nc.sync.dma_start(
                out=scr_b.rearrange("o (b p) -> (o b) p", p=P)[d0:d0 + nr, :],
                in_=bT[r0:r1, 0, :])

        with ExitStack() as s1:
            kv1 = s1.enter_context(tc.tile_pool(name="kv1", bufs=1))
            xs_pool = s1.enter_context(tc.tile_pool(name="xs", bufs=2))
            st_pool = s1.enter_context(tc.tile_pool(name="stw", bufs=2))
            att_pool = s1.enter_context(tc.tile_pool(name="att", bufs=3))
            pv_pool = s1.enter_context(
                tc.tile_pool(name="pv", bufs=1, space="PSUM"))
            ps_pool = s1.enter_context(
                tc.tile_pool(name="ps", bufs=2, space="PSUM"))
            pb_pool = s1.enter_context(
                tc.tile_pool(name="pb", bufs=2, space="PSUM"))

            killd_t = kv1.tile([P, QB * 8 * P], BF16)
            a_blk = kv1.tile([P, P], BF16)
            b_blk = kv1.tile([P, P], BF16)
            mv_blk = kv1.tile([P, 40, 2], F32)
            aq_b = kv1.tile([P, QB * P], BF16)
            wo_sb = kv1.tile([P, DT, D], BF16)
            cso_sb = kv1.tile([32, 2, D], FP8)
            attn_T = kv1.tile([P, DT, QB, P], BF16)

            xa_scope = ExitStack()
            xap = xa_scope.enter_context(tc.tile_pool(name="xap", bufs=1))
            xt_pool = xa_scope.enter_context(tc.tile_pool(name="xtp", bufs=2))
            wk8_sb = xap.tile([P, DT, D], FP8)
            nc.sync.dma_start(out=wk8_sb,
                              in_=wk8_d.rearrange("(u p) o -> p u o", p=P))
            wv8_sb = xap.tile([P, DT, D], FP8)
            nc.sync.dma_start(out=wv8_sb,
                              in_=wv8_d.rearrange("(u p) o -> p u o", p=P))
            csk_sb = xap.tile([32, 2, D], FP8)
            nc.sync.dma_start(out=csk_sb, in_=csk_d)
            csv_sb = xap.tile([32, 2, D], FP8)
            nc.sync.dma_start(out=csv_sb, in_=csv_d)
            a_b = xap.tile([P, S], BF16)
            b_pad = xap.tile([32, 2, S], FP8)
            nc.sync.dma_start(out=b_pad, in_=bones_d)
            xa8 = xap.tile([P, DT, S], FP8)

            # ---- own-row stats + Q projection ----
            xq_blk = xap.tile([P, QB, D], BF16)
            nc.sync.dma_start(out=xq_blk,
                              in_=xq_d.rearrange("(r p) f -> p r f", p=P))
            for i in range(QB):
                bn_block(st_pool, xq_blk[:, i, :], mv_blk[:, 32 + i, :])
            rstd_batch(st_pool, mv_blk[:, 32:36, :], a_blk[:, 32:36],
                       b_blk[:, 32:36])
            bounce(st_pool, a_blk, b_blk, scraq_d, scrbq_d, 32, 36, d0=0)
            nc.sync.dma_start(out=aq_b, in_=scraq_d.to_broadcast([P, QB * P]))
            nc.gpsimd.dma_start(out=bq_pad[0:1, 0, :], in_=scrbq_d)
            xqa8 = xap.tile([P, DT, QB * P], FP8)
            for u in range(DT):
                nc.vector.scalar_tensor_tensor(
                    out=xqa8[:, u, :], in0=xqT_sb[:, u, :], scalar=1.0,
                    in1=aq_b, op0=MUL, op1=MUL)
            wq8_sb = xap.tile([P, DT, D], FP8)
            nc.sync.dma_start(out=wq8_sb,
                              in_=wq8_d.rearrange("(u p) o -> p u o", p=P))
            csq_sb = xap.tile([32, 2, D], FP8)
            nc.sync.dma_start(out=csq_sb, in_=csq_d)
            for cidx in range(6):
                t, slot = divmod(cidx, 3)
                pmt = ps_pool.tile([P, 8 * P], F32, tag="ps8", name="pmt")
                pm = pmt[:, 0:QB * P]
                for g in range(3):
                    nc.tensor.matmul(pm, wq8_sb[:, 2 * g:2 * g + 2,
                                                cidx * P:(cidx + 1) * P],
                                     xqa8[:, 2 * g:2 * g + 2, :],
                                     start=(g == 0), stop=False, perf_mode=DRM)
                nc.tensor.matmul(pm, csq_sb[:, :, cidx * P:(cidx + 1) * P],
                                 bq_pad, start=False, stop=True, perf_mode=DRM,
                                 skip_group_check=True)
                nc.vector.tensor_copy(out=q8[:, t, slot, :], in_=pm)

            def build_units(sb):
                c0, c1 = sb * 512, (sb + 1) * 512
                us = []

                def xa_unit(u0):
                    def f():
                        for u in (u0, u0 + 1):
                            xab = xt_pool.tile([P, 512], BF16, tag="xab")
                            nc.vector.tensor_mul(out=xab, in0=xts[sb][:, u, :],
                                                 in1=a_b[:, c0:c1])
                            nc.gpsimd.tensor_copy(out=xa8[:, u, c0:c1],
                                                  in_=xab)
                    return f

                def k_unit(cidx):
                    def f():
                        t, slot = divmod(cidx, 3)
                        pm = pv_pool.tile([P, 512], F32, tag="pv1",
                                          name="pm")
                        for g in range(3):
                            nc.tensor.matmul(
                                pm, wk8_sb[:, 2 * g:2 * g + 2,
                                           cidx * P:(cidx + 1) * P],
                                xa8[:, 2 * g:2 * g + 2, c0:c1],
                                start=(g == 0), stop=False, perf_mode=DRM)
                        nc.tensor.matmul(
                            pm, csk_sb[:, :, cidx * P:(cidx + 1) * P],
                            b_pad[:, :, c0:c1], start=False, stop=True,
                            perf_mode=DRM, skip_group_check=True)
                        if cidx % 2 == 0:
                            nc.scalar.activation(out=k8[:, t, slot, c0:c1],
                                                 in_=pm, func=AF.Identity,
                                                 scale=1.0)
                        else:
                            nc.vector.tensor_copy(out=k8[:, t, slot, c0:c1],
                                                  in_=pm)
                    return f

                def v_unit(r):
                    def f():
                        blk = sb * 4 + r
                        bc0 = blk * P
                        pm1 = pv_pool.tile([P, 512], F32, tag="pv1")
                        pm2 = pv_pool.tile([P, 256], F32, tag="pv2")
                        for pm, cols in ((pm1, slice(0, 512)),
                                         (pm2, slice(512, 768))):
                            for g in range(3):
                                nc.tensor.matmul(
                                    pm,
                                    xa8[:, 2 * g:2 * g + 2, bc0:bc0 + P],
                                    wv8_sb[:, 2 * g:2 * g + 2, cols],
                                    start=(g == 0), stop=False, perf_mode=DRM)
                            nc.tensor.matmul(
                                pm, b_pad[:, :, bc0:bc0 + P],
                                csv_sb[:, :, cols], start=False, stop=True,
                                perf_mode=DRM, skip_group_check=True)
                        nc.vector.tensor_copy(
                            out=v8[:, blk, 0:8, 0:HD],
                            in_=pm1.rearrange("p (h d) -> p h d", h=8))
                        nc.vector.tensor_copy(
                            out=v8[:, blk, 8:12, 0:HD],
                            in_=pm2.rearrange("p (h d) -> p h d", h=4))
                    return f

                for u0 in range(0, DT, 2):
                    us.append(xa_unit(u0))
                for cidx in range(6):
                    us.append(k_unit(cidx))
                for r in range(4):
                    us.append(v_unit(r))
                return us

            def attn_i(i, units=()):
                nj = 8 * i + 8
                units = list(units)
                slots = 6 * (i + 1)
                nunits = len(units)
                slot_no = [0]

                def fill():
                    # Bresenham pacing: after slot k, ~(k+1)*n/slots units out
                    slot_no[0] += 1
                    target = (slot_no[0] * nunits + slots - 1) // slots
                    while units and nunits - len(units) < target:
                        units.pop(0)()

                def sc_part(h, g):
                    q0 = 32 * (h % 4)
                    hs = h // 4
                    ps8 = ps_pool.tile([P, 8 * P], F32, tag="ps8", name="ps8")
                    for jj in range(8):
                        j = 8 * g + jj
                        nc.tensor.matmul(
                            ps8[:, jj * P:(jj + 1) * P],
                            k8[q0:q0 + 32, :, hs, j * P:(j + 1) * P],
                            q8[q0:q0 + 32, :, hs, i * P:(i + 1) * P],
                            start=True, stop=True, perf_mode=DRM,
                            tile_position=(q0, 0))
                    return ps8

                def ea_part(h, g, ps8, first, last):
                    sl = 0
                    vals = vals_t[h]
                    if g == i:   # diagonal group: mask, bf16 AV
                        pT = att_pool.tile([P, 8 * P], BF16, tag="pTb")
                        nc.scalar.activation(out=pT, in_=ps8, func=AF.Exp,
                                             scale=ESC)
                        eng = nc.vector if (h % 2 == 0) else nc.gpsimd
                        eng.tensor_mul(
                            out=pT, in0=pT,
                            in1=killd_t[:, i * 1024:(i + 1) * 1024])
                        for jj in range(8):
                            nc.tensor.matmul(
                                vals[:, sl:sl + HD1],
                                pT[:, jj * P:(jj + 1) * P],
                                v8[:, 8 * g + jj, h, :],
                                start=(first and jj == 0),
                                stop=(last and jj == 7),
                                skip_group_check=True)
                    else:        # fully-live group: fp8 probs, DR AV
                        pT8 = att_pool.tile([P, 8 * P], FP8, tag="pT8")
                        nc.scalar.activation(out=pT8, in_=ps8,
                                             func=AF.Exp, scale=ESC)
                        p2 = pT8.rearrange("p (m q) -> p m q", m=8)
                        for m in range(4):
                            nc.tensor.matmul(
                                vals[:, sl:sl + HD1],
                                p2[:, 2 * m:2 * m + 2, :],
                                v8[:, 8 * g + 2 * m:8 * g + 2 * m + 2, h, :],
                                start=(first and m == 0),
                                stop=(last and m == 3), perf_mode=DRM,
                                skip_group_check=True)

                def drain_part(h):
                    sl = 0
                    vals = vals_t[h]
                    rs = att_pool.tile([P, 1], F32, tag="rs")
                    nc.vector.reciprocal(out=rs, in_=vals[:, sl + HD:sl + HD1])
                    nc.vector.tensor_scalar_mul(
                        out=attn_sb[:, i, h * HD:(h + 1) * HD],
                        in0=vals[:, sl:sl + HD], scalar1=rs)

                vals_t = {}
                gorder = [i] + list(range(i))   # diagonal group first
                for hp in range(0, H, 2):
                    vals_t[hp] = pb_pool.tile([P, 512], F32, tag="vals",
                                              name="vals")
                    vals_t[hp + 1] = pb_pool.tile([P, 512], F32, tag="vals",
                                                  name="vals")
                    h0, h1 = hp, hp + 1
                    for n, g in enumerate(gorder):
                        psa = sc_part(h0, g)
                        psb = sc_part(h1, g)
                        fill()
                        ea_part(h0, g, psa, n == 0, n == i)
                        ea_part(h1, g, psb, n == 0, n == i)
                    drain_part(h0)
                    drain_part(h1)
                while units:
                    units.pop(0)()

            def wo_part(i):
                """Wo + x1T for query block i (pipelined after attn_i(i))."""
                nc.sync.dma_start_transpose(out=attn_T[:, :, i, :],
                                            in_=attn_sb[:, i, :])
                pw1 = pv_pool.tile([P, 512], F32, tag="pv1")
                pw2 = pv_pool.tile([P, 256], F32, tag="pv2")
                for ot in range(6):
                    pm = pw1[:, ot * P:(ot + 1) * P] if ot < 4 else \
                        pw2[:, (ot - 4) * P:(ot - 3) * P]
                    for dt in range(DT):
                        nc.tensor.matmul(
                            pm,
                            wo_sb[:, dt, ot * P:(ot + 1) * P],
                            attn_T[:, dt, i, :], start=(dt == 0), stop=False,
                            skip_group_check=True)
                    nc.tensor.matmul(pm,
                                     cso_sb[:, :, ot * P:(ot + 1) * P],
                                     bq_pad[:, :, i * P:(i + 1) * P],
                                     start=False, stop=True, perf_mode=DRM,
                                     skip_group_check=True)
                nc.vector.tensor_add(
                    out=x1T[:, 0:4, i * P:(i + 1) * P],
                    in0=pw1.rearrange("p (u c) -> p u c", u=4),
                    in1=xqT_sb[:, 0:4, i * P:(i + 1) * P])
                nc.vector.tensor_add(
                    out=x1T[:, 4:6, i * P:(i + 1) * P],
                    in0=pw2.rearrange("p (u c) -> p u c", u=2),
                    in1=xqT_sb[:, 4:6, i * P:(i + 1) * P])
                x1r = st_pool.tile([P, DT, P], BF16, tag="x1r")
                for ot in range(6):
                    nc.sync.dma_start_transpose(
                        out=x1r[:, ot, :], in_=x1T[:, ot, i * P:(i + 1) * P])
                bn_block(st_pool, x1r.rearrange("p u c -> p (u c)"),
                         mv_blk[:, 36 + i, :])

            # ---- stats + build + attention pipeline: builds/stats of
            # later rounds are interleaved INTO the attention emission so no
            # engine queue ever parks behind another engine's work ----
            xts = {}

            def stats_units(half):
                us = []

                def sb_unit(sb):
                    def f():
                        xs = xs_pool.tile([P, 4, D], BF16, tag="xs")
                        nc.sync.dma_start(
                            out=xs,
                            in_=xrow_d[sb * 512:(sb + 1) * 512, :].rearrange(
                                "(r p) f -> p r f", p=P))
                        for r in range(4):
                            bn_block(st_pool, xs[:, r, :],
                                     mv_blk[:, sb * 4 + r, :])
                        xt = xt_pool.tile([P, DT, 512], BF16, tag="xt")
                        nc.sync.dma_start(
                            out=xt,
                            in_=xT_d.rearrange("(u p) s -> p u s", p=P)[
                                :, :, sb * 512:(sb + 1) * 512])
                        xts[sb] = xt
                    return f

                for sb in (2 * half, 2 * half + 1):
                    us.append(sb_unit(sb))
                return us

            def rstd_rows(r0, r1):
                n = r1 - r0
                rstd_batch(st_pool, mv_blk[:, r0:r1, :],
                           a_blk[:, r0:r1], b_blk[:, r0:r1])
                bounce(st_pool, a_blk, b_blk, scra_d, scrb_d, r0, r1, d0=r0)
                nc.sync.dma_start(
                    out=a_b[:, r0 * P:r1 * P],
                    in_=scra_d[:, r0 * P:r1 * P].to_broadcast([P, n * P]))
                nc.gpsimd.dma_start(out=b_pad[0:1, 0, r0 * P:r1 * P],
                                    in_=scrb_d[:, r0 * P:r1 * P])

            def rstd_bcast(half):
                rstd_rows(half * 8, half * 8 + 8)

            su0 = stats_units(0)
            su0[0]()                 # sb0 stats
            rstd_rows(0, 4)
            b0 = build_units(0)
            for u in b0[0:3]:        # xa8(sb0)
                u()
            su0[1]()                 # sb1 stats
            rstd_rows(4, 8)
            for u in b0[3:] + build_units(1):
                u()
            for u in stats_units(1):
                u()
            rstd_bcast(1)

            nc.sync.dma_start(out=killd_t, in_=killd_d)
            nc.sync.dma_start(out=wo_sb,
                              in_=wo_d.rearrange("(u p) o -> p u o", p=P))
            nc.sync.dma_start(out=cso_sb, in_=cso_d)
            attn_i(0, build_units(2) + build_units(3) + stats_units(2))
            wo_part(0)
            rstd_bcast(2)
            attn_i(1, build_units(4) + build_units(5) + stats_units(3))
            wo_part(1)
            rstd_bcast(3)
            attn_i(2, build_units(6) + build_units(7))
            wo_part(2)

            xa_scope.close()

            # FFN1 weights load during the i=3 attention tail
            fw = s1.enter_context(tc.tile_pool(name="fw", bufs=1))
            fw1_scope = ExitStack()
            fw1 = fw1_scope.enter_context(tc.tile_pool(name="fw1", bufs=1))
            w18_sb = fw1.tile([P, DT, HID], FP8)
            nc.sync.dma_start(out=w18_sb,
                              in_=w18_d.rearrange("(u p) o -> p u o", p=P))
            w18lo_sb = fw1.tile([P, DT, HID], FP8)
            nc.sync.dma_start(out=w18lo_sb,
                              in_=w18lo_d.rearrange("(u p) o -> p u o", p=P))
            csw1_sb = fw1.tile([32, 2, HID], FP8)
            nc.sync.dma_start(out=csw1_sb, in_=csw1_d)

            attn_i(3)
            wo_part(3)

            # ---- LN2 rstd + x1a8 ----
            rstd_batch(st_pool, mv_blk[:, 36:40, :], a_blk[:, 36:40],
                       b_blk[:, 36:40])
            bounce(st_pool, a_blk, b_blk, scra2_d, scrb2_d, 36, 40, d0=0)
            a2_b = fw.tile([P, QB * P], BF16)
            nc.sync.dma_start(out=a2_b, in_=scra2_d.to_broadcast([P, QB * P]))
            b2_pad = fw.tile([32, 2, QB * P], FP8)
            nc.sync.dma_start(out=b2_pad, in_=bones_d[:, :, 0:QB * P])
            nc.gpsimd.dma_start(out=b2_pad[0:1, 0, :], in_=scrb2_d)
            x1a8 = fw.tile([P, DT, QB * P], FP8)
            for u in range(DT):
                nc.vector.scalar_tensor_tensor(
                    out=x1a8[:, u, :], in0=x1T[:, u, :], scalar=1.0,
                    in1=a2_b, op0=MUL, op1=MUL)

            # ---- FFN ----
            h8 = fw.tile([P, 2, FT // 2, QB * P], FP8)
            for ft in range(FT):
                pmt = ps_pool.tile([P, 8 * P], F32, tag="ps8", name="pmt")
                pm = pmt[:, 0:QB * P]
                for g in range(3):
                    nc.tensor.matmul(pm, w18_sb[:, 2 * g:2 * g + 2,
                                                ft * P:(ft + 1) * P],
                                     x1a8[:, 2 * g:2 * g + 2, :],
                                     start=(g == 0), stop=False, perf_mode=DRM)
                for g in range(3):
                    nc.tensor.matmul(pm, w18lo_sb[:, 2 * g:2 * g + 2,
                                                  ft * P:(ft + 1) * P],
                                     x1a8[:, 2 * g:2 * g + 2, :],
                                     start=False, stop=False, perf_mode=DRM,
                                     skip_group_check=True)
                nc.tensor.matmul(pm, csw1_sb[:, :, ft * P:(ft + 1) * P],
                                 b2_pad, start=False, stop=True,
                                 perf_mode=DRM, skip_group_check=True)
                nc.scalar.activation(out=h8[:, ft % 2, ft // 2, :], in_=pm,
                                     func=AF.Gelu, scale=1.0 / KS)
            fw1_scope.close()
            fw2 = s1.enter_context(tc.tile_pool(name="fw2", bufs=1))
            csw2_sb = fw2.tile([32, 2, D], FP8)
            nc.sync.dma_start(out=csw2_sb, in_=csw2_d)
            w28_sb = fw2.tile([P, 2, FT // 2, D], FP8)
            nc.sync.dma_start(
                out=w28_sb.rearrange("p q v o -> p (q v) o"),
                in_=w28_d.rearrange("(u p) o -> p u o", p=P))
            w28lo_sb = fw2.tile([P, 2, FT // 2, D], FP8)
            nc.sync.dma_start(
                out=w28lo_sb.rearrange("p q v o -> p (q v) o"),
                in_=w28lo_d.rearrange("(u p) o -> p u o", p=P))
            outT = fw.tile([P, DT, QB * P], BF16)
            for ot in range(6):
                pmt = ps_pool.tile([P, 8 * P], F32, tag="ps8", name="pmt")
                pm = pmt[:, 0:QB * P]
                for v in range(FT // 2):
                    nc.tensor.matmul(pm, w28_sb[:, :, v, ot * P:(ot + 1) * P],
                                     h8[:, :, v, :], start=(v == 0),
                                     stop=False, perf_mode=DRM)
                for v in range(FT // 2):
                    nc.tensor.matmul(pm,
                                     w28lo_sb[:, :, v, ot * P:(ot + 1) * P],
                                     h8[:, :, v, :], start=False,
                                     stop=False, perf_mode=DRM,
                                     skip_group_check=True)
                nc.tensor.matmul(pm, csw2_sb[:, :, ot * P:(ot + 1) * P],
                                 bq_pad, start=False, stop=True,
                                 perf_mode=DRM, skip_group_check=True)
                nc.vector.scalar_tensor_tensor(
                    out=outT[:, ot, :], in0=pm, scalar=1.0 / WS,
                    in1=x1T[:, ot, :], op0=MUL, op1=ADD)
                o_r = st_pool.tile([P, QB, P], BF16, tag="or")
                nc.sync.dma_start_transpose(out=o_r, in_=outT[:, ot, :])
                nc.sync.dma_start(
                    out=out_d.rearrange("(r p) f -> p r f", p=P)[
                        :, :, ot * P:(ot + 1) * P],
                    in_=o_r)

    nc.compile()
    return nc


def _kq_perm():
    """Column permutation for Wk/Wq: tile cidx = t*3 + slot, col j ->
    original feature 64*h + d with h = slot*4 + j//32, d = t*32 + j%32."""
    perm = np.zeros(D, np.int64)
    for cidx in range(6):
        t, slot = divmod(cidx, 3)
        for j in range(P):
            h = slot * 4 + j // 32
            d = t * 32 + (j % 32)
            perm[cidx * P + j] = 64 * h + d
    return perm


def _f8(x):
    return np.clip(np.asarray(x, np.float64), -240.0, 240.0).astype(f8dt)


def _cs_pack(cs_row, bias_row, n):
    """[32, 2, n] fp8 rank-1 lhsT: (p=0,t=0) = cs_row, (p=1,t=1) = bias_row."""
    m = np.zeros((32, 2, n), np.float64)
    m[0, 0, :] = cs_row
    m[1, 1, :] = bias_row
    return _f8(m)


def _prep_inputs(x, gamma1, beta1, Wqkv, bqkv, Wo, bo, gamma2, beta2,
                 W1, b1, W2, b2):
    x2 = np.asarray(x, np.float64).reshape(S, D)
    g1 = np.asarray(gamma1, np.float64)
    be1 = np.asarray(beta1, np.float64)
    g2 = np.asarray(gamma2, np.float64)
    be2 = np.asarray(beta2, np.float64)
    W4 = np.asarray(Wqkv, np.float64).reshape(D, H, 3, HD)
    b4 = np.asarray(bqkv, np.float64).reshape(H, 3, HD)
    wq = W4[:, :, 0, :].reshape(D, D)
    wk = W4[:, :, 1, :].reshape(D, D)
    wv = W4[:, :, 2, :].reshape(D, D)
    bq = b4[:, 0, :].reshape(D)
    bk = b4[:, 1, :].reshape(D)
    bv = b4[:, 2, :].reshape(D)
    Wo = np.asarray(Wo, np.float64)
    bo = np.asarray(bo, np.float64)
    W1 = np.asarray(W1, np.float64)
    b1 = np.asarray(b1, np.float64)
    W2 = np.asarray(W2, np.float64)
    b2 = np.asarray(b2, np.float64)

    def _w2p(W):
        return WS * W.reshape(FT // 2, 2, P, D).transpose(1, 0, 2, 3).reshape(
            HID, D)

    perm = _kq_perm()
    wq_t = g1[:, None] * wq
    wk_t = g1[:, None] * wk
    wv_t = g1[:, None] * wv
    bq_t = bq + be1 @ wq
    bk_t = bk + be1 @ wk
    bv_t = bv + be1 @ wv
    w1_t = g2[:, None] * W1
    b1_t = b1 + be2 @ W1

    bones = np.zeros((32, 2, S), f8dt)
    bones[1, 1, :] = f8dt(1.0)

    common = {
        "xrow": x2.astype(bfdt),
        "xT": np.ascontiguousarray(x2.T).astype(bfdt),
        "wk8": _f8(WS * wk_t[:, perm]),
        "wq8": _f8(WS * wq_t[:, perm]),
        "wv8": _f8(WS * wv_t),
        "wo": np.ascontiguousarray(Wo / KS).astype(bfdt),
        "w18": _f8(WS * w1_t),
        "w18lo": _f8(WS * w1_t - _f8(WS * w1_t).astype(np.float64)),
        "w28": _f8(_w2p(W2)),
        "w28lo": _f8(_w2p(W2) - _f8(_w2p(W2)).astype(np.float64)),
        "csk": _cs_pack(WS * wk_t.sum(0)[perm], KS * bk_t[perm], D),
        "csq": _cs_pack(WS * wq_t.sum(0)[perm], KS * bq_t[perm], D),
        "csv": _cs_pack(WS * wv_t.sum(0), KS * bv_t, D),
        "csw1": _cs_pack(WS * w1_t.sum(0), KS * b1_t, HID),
        "csw2": _cs_pack(np.zeros(D), WS * b2, D),
        "cso": _cs_pack(np.zeros(D), bo, D),
        "bones": bones,
    }

    xb = x2.reshape(NB, P, D)
    tri_T = np.tril(np.ones((P, P), np.float64)).T  # [k, q] k<=q
    in_maps = []
    for c in range(NCORES):
        blocks = [c + 8 * i for i in range(QB)]
        xq = np.ascontiguousarray(xb[blocks].reshape(QB * P, D))
        killd = np.zeros((P, QB * 8 * P), np.float64)
        for i in range(QB):
            for jj in range(8):
                t0 = (i * 8 + jj) * P
                if jj < c:
                    killd[:, t0:t0 + P] = 1.0
                elif jj == c:
                    killd[:, t0:t0 + P] = tri_T
        m = dict(common)
        m["xq"] = xq.astype(bfdt)
        m["xqT"] = np.ascontiguousarray(xq.T).astype(bfdt)
        m["killd"] = killd.astype(bfdt)
        in_maps.append(m)
    return in_maps


def kernel(**inputs):
    nc = _CACHE.get("nc")
    if nc is None:
        nc = _build()
        _CACHE["nc"] = nc
    in_maps = _prep_inputs(**inputs)
    res = run_bass_kernel_spmd(nc, in_maps, list(range(NCORES)))
    out = np.zeros((S, D), np.float32)
    ob = out.reshape(NB, P, D)
    for c in range(NCORES):
        o = np.asarray(res.results[c]["out"], dtype=np.float32).reshape(
            QB, P, D)
        for i in range(QB):
            ob[c + 8 * i] = o[i]
    return out.reshape(1, S, D)



# revision 7
# speedup vs baseline: 1.0327x; 1.0327x over previous
"""Trainium2 Bass kernel v2 for a GPT decoder block (d=768, H=12, S=4096, FFN=3072).

Sharding: 8-way SPMD over query-row blocks, cyclic (core c owns blocks
{c, c+8, c+16, c+24}).  Every core builds K/V for the full sequence.

Design (vs the v1 baseline):
- LayerNorm folded into the projections: host folds gamma/beta into weights
  (W~ = diag(gamma) W, bias += beta^T W); the device computes per-row
  a = rstd/2, b = -mu*rstd/2; normalization is one fused DVE multiply
  (xa8 = x^T * a_bcast -> fp8) plus a rank-1 correction row in each matmul.
- fp8(e4m3) DoubleRow matmuls (2x PE) for Q/K/V/FFN/scores.  Weights
  prescaled by 32, activations by 1/2, so PSUM lands in drain units; biases
  enter as rank-1 rows against a resident [b; ones] operand, making drains
  pure casts.  FFN weights use a hi+lo fp8 split (~11-bit effective).
- rstd via exp(-0.5 ln(var+eps)): Ln/Exp share an activation table set with
  the attention Exp, so there are no act-table reloads in the main pipeline.
- Scores are 32-partition DoubleRow fp8; AV uses fp8 probs + DoubleRow for
  fully-live groups, bf16 probs for the diagonal group (the only masked one).
- DMA-engine (xbar) transposes for all layout changes; per-2-superblock
  stats/K/V/attention pipelining; Wo + LN2 pipelined per query block.
"""

import os
import sys
from contextlib import ExitStack

import numpy as np
import ml_dtypes

for _p in ("/opt/trn_rl_repo", "/opt/pypackages"):
    if os.path.isdir(_p) and _p not in sys.path:
        sys.path.append(_p)

import concourse.bacc as bacc
import concourse.tile as tile
from concourse import mybir
from concourse.bass_utils import run_bass_kernel_spmd

P = 128
D = 768
DT = 6
H = 12
HD = 64
S = 4096
NB = 32
SB = 8
HID = 3072
FT = 24
QB = 4
NCORES = 8
EPS = 1e-5
USE_LO1 = False          # fp8 hi+lo split for W1 (False: plain fp8)
USE_LO2 = False          # fp8 hi+lo split for W2
SCALE = 0.125
XS = 0.5                 # activation prescale (a carries XS*rstd)
WS = 32.0                # weight fp8 prescale
KS = XS * WS             # = 16: scale of k8/q8/v8
ESC = SCALE / (KS * KS)  # exp() scale on score psum
HD1 = HD + 1

F32 = mybir.dt.float32
BF16 = mybir.dt.bfloat16
FP8 = mybir.dt.float8e4
AF = mybir.ActivationFunctionType
DRM = mybir.MatmulPerfMode.DoubleRow
MUL = mybir.AluOpType.mult
ADD = mybir.AluOpType.add
SHR = mybir.AluOpType.logical_shift_right
XOR = mybir.AluOpType.bitwise_xor
I32 = mybir.dt.int32

bfdt = ml_dtypes.bfloat16
f8dt = ml_dtypes.float8_e4m3

_CACHE = {}


def _build():
    nc = bacc.Bacc("TRN2", target_bir_lowering=False, debug=False,
                   num_devices=NCORES)

    # ---- DRAM I/O ----
    xrow_d = nc.dram_tensor("xrow", [S, D], BF16, kind="ExternalInput").ap()
    xT_d = nc.dram_tensor("xT", [D, S], BF16, kind="ExternalInput").ap()
    xq_d = nc.dram_tensor("xq", [QB * P, D], BF16, kind="ExternalInput").ap()
    xqT_d = nc.dram_tensor("xqT", [D, QB * P], BF16, kind="ExternalInput").ap()
    wk8_d = nc.dram_tensor("wk8", [D, D], FP8, kind="ExternalInput").ap()
    wq8_d = nc.dram_tensor("wq8", [D, D], FP8, kind="ExternalInput").ap()
    wv8_d = nc.dram_tensor("wv8", [D, D], FP8, kind="ExternalInput").ap()
    wo_d = nc.dram_tensor("wo", [D, D], BF16, kind="ExternalInput").ap()
    w18_d = nc.dram_tensor("w18", [D, HID], FP8, kind="ExternalInput").ap()
    w28_d = nc.dram_tensor("w28", [HID, D], FP8, kind="ExternalInput").ap()
    w18lo_d = nc.dram_tensor("w18lo", [D, HID], FP8, kind="ExternalInput").ap()
    w28lo_d = nc.dram_tensor("w28lo", [HID, D], FP8, kind="ExternalInput").ap()
    csk_d = nc.dram_tensor("csk", [32, 2, D], FP8, kind="ExternalInput").ap()
    csq_d = nc.dram_tensor("csq", [32, 2, D], FP8, kind="ExternalInput").ap()
    csv_d = nc.dram_tensor("csv", [32, 2, D], FP8, kind="ExternalInput").ap()
    csw1_d = nc.dram_tensor("csw1", [32, 2, HID], FP8, kind="ExternalInput").ap()
    csw2_d = nc.dram_tensor("csw2", [32, 2, D], FP8, kind="ExternalInput").ap()
    cso_d = nc.dram_tensor("cso", [32, 2, D], FP8, kind="ExternalInput").ap()
    killd_d = nc.dram_tensor("killd", [P, QB * 8 * P], BF16,
                             kind="ExternalInput").ap()
    bones_d = nc.dram_tensor("bones", [32, 2, S], FP8,
                             kind="ExternalInput").ap()
    out_d = nc.dram_tensor("out", [QB * P, D], BF16, kind="ExternalOutput").ap()
    scra_d = nc.dram_tensor("scra", [1, S], BF16, kind="Internal").ap()
    scrb_d = nc.dram_tensor("scrb", [1, S], BF16, kind="Internal").ap()
    scraq_d = nc.dram_tensor("scraq", [1, QB * P], BF16, kind="Internal").ap()
    scrbq_d = nc.dram_tensor("scrbq", [1, QB * P], BF16, kind="Internal").ap()
    scra2_d = nc.dram_tensor("scra2", [1, QB * P], BF16, kind="Internal").ap()
    scrb2_d = nc.dram_tensor("scrb2", [1, QB * P], BF16, kind="Internal").ap()

    with tile.TileContext(nc) as tc, ExitStack() as root:
        singles = root.enter_context(tc.tile_pool(name="singles", bufs=1))

        eps_t = singles.tile([P, 1], F32)
        nc.vector.memset(eps_t, EPS)
        lnxs_t = singles.tile([P, 1], F32)
        nc.vector.memset(lnxs_t, float(np.log(XS)))

        xqT_sb = singles.tile([P, DT, QB * P], BF16)
        nc.sync.dma_start(out=xqT_sb,
                          in_=xqT_d.rearrange("(u p) s -> p u s", p=P))
        attn_sb = singles.tile([P, QB, D], BF16)
        x1T = singles.tile([P, DT, QB * P], BF16)
        q8 = singles.tile([P, 2, 3, QB * P], FP8)
        k8 = singles.tile([P, 2, 3, S], FP8)
        v8 = singles.tile([P, NB, H, HD1], FP8)
        nc.vector.memset(v8[:, :, :, HD:HD1], 1.0)
        bq_pad = singles.tile([32, 2, QB * P], FP8)
        nc.sync.dma_start(out=bq_pad, in_=bones_d[:, :, 0:QB * P])

        def bn_block(pool, src_ap, mv_dst):
            stats = pool.tile([P, 3, nc.vector.BN_STATS_DIM], F32, tag="st")
            xg = src_ap.rearrange("p (g f) -> p g f", g=3)
            for g in range(3):
                nc.vector.bn_stats(out=stats[:, g, :], in_=xg[:, g, :])
            nc.vector.bn_aggr(out=mv_dst, in_=stats)

        def rstd_batch(pool, mv_cols, a_cols, b_cols):
            """[P, n, 2] (mean,var) -> a [P,n] = XS*rstd, b = -XS*mu*rstd."""
            n = mv_cols.shape[1]
            lnz = pool.tile([P, n], F32, tag="lnz")
            nc.scalar.activation(out=lnz, in_=mv_cols[:, :, 1], func=AF.Ln,
                                 bias=eps_t, scale=1.0)
            a_f = pool.tile([P, n], F32, tag="af")
            nc.scalar.activation(out=a_f, in_=lnz, func=AF.Exp,
                                 bias=lnxs_t, scale=-0.5)
            nc.vector.tensor_copy(out=a_cols, in_=a_f)
            nc.vector.scalar_tensor_tensor(out=b_cols, in0=mv_cols[:, :, 0],
                                           scalar=-1.0, in1=a_f,
                                           op0=MUL, op1=MUL)

        def bounce(pool, a_blk, b_blk, scr_a, scr_b, r0, r1, d0=0):
            nr = r1 - r0
            aT = pool.tile([P, 1, P], BF16, tag="aT")
            nc.sync.dma_start_transpose(out=aT, in_=a_blk)
            nc.sync.dma_start(
                out=scr_a.rearrange("o (b p) -> (o b) p", p=P)[d0:d0 + nr, :],
                in_=aT[r0:r1, 0, :])
            bT = pool.tile([P, 1, P], BF16, tag="bT")
            nc.sync.dma_start_transpose(out=bT, in_=b_blk)
            nc.sync.dma_start(
                out=scr_b.rearrange("o (b p) -> (o b) p", p=P)[d0:d0 + nr, :],
                in_=bT[r0:r1, 0, :])

        with ExitStack() as s1:
            kv1 = s1.enter_context(tc.tile_pool(name="kv1", bufs=1))
            xs_pool = s1.enter_context(tc.tile_pool(name="xs", bufs=2))
            st_pool = s1.enter_context(tc.tile_pool(name="stw", bufs=2))
            att_pool = s1.enter_context(tc.tile_pool(name="att", bufs=3))
            pv_pool = s1.enter_context(
                tc.tile_pool(name="pv", bufs=1, space="PSUM"))
            ps_pool = s1.enter_context(
                tc.tile_pool(name="ps", bufs=2, space="PSUM"))
            pb_pool = s1.enter_context(
                tc.tile_pool(name="pb", bufs=2, space="PSUM"))

            killd_t = kv1.tile([P, QB * 8 * P], BF16)
            a_blk = kv1.tile([P, P], BF16)
            b_blk = kv1.tile([P, P], BF16)
            mv_blk = kv1.tile([P, 40, 2], F32)
            aq_b = kv1.tile([P, QB * P], BF16)
            wo_sb = kv1.tile([P, DT, D], BF16)
            cso_sb = kv1.tile([32, 2, D], FP8)
            attn_T = kv1.tile([P, DT, QB, P], BF16)

            xa_scope = ExitStack()
            xap = xa_scope.enter_context(tc.tile_pool(name="xap", bufs=1))
            xt_pool = xa_scope.enter_context(tc.tile_pool(name="xtp", bufs=2))
            wk8_sb = xap.tile([P, DT, D], FP8)
            nc.sync.dma_start(out=wk8_sb,
                              in_=wk8_d.rearrange("(u p) o -> p u o", p=P))
            wv8_sb = xap.tile([P, DT, D], FP8)
            nc.sync.dma_start(out=wv8_sb,
                              in_=wv8_d.rearrange("(u p) o -> p u o", p=P))
            csk_sb = xap.tile([32, 2, D], FP8)
            nc.sync.dma_start(out=csk_sb, in_=csk_d)
            csv_sb = xap.tile([32, 2, D], FP8)
            nc.sync.dma_start(out=csv_sb, in_=csv_d)
            a_b = xap.tile([P, S], BF16)
            b_pad = xap.tile([32, 2, S], FP8)
            nc.sync.dma_start(out=b_pad, in_=bones_d)
            xa8 = xap.tile([P, DT, S], FP8)

            # ---- own-row stats + Q projection ----
            xq_blk = xap.tile([P, QB, D], BF16)
            nc.sync.dma_start(out=xq_blk,
                              in_=xq_d.rearrange("(r p) f -> p r f", p=P))
            for i in range(QB):
                bn_block(st_pool, xq_blk[:, i, :], mv_blk[:, 32 + i, :])
            rstd_batch(st_pool, mv_blk[:, 32:36, :], a_blk[:, 32:36],
                       b_blk[:, 32:36])
            bounce(st_pool, a_blk, b_blk, scraq_d, scrbq_d, 32, 36, d0=0)
            nc.sync.dma_start(out=aq_b, in_=scraq_d.to_broadcast([P, QB * P]))
            nc.gpsimd.dma_start(out=bq_pad[0:1, 0, :], in_=scrbq_d)
            xqa8 = xap.tile([P, DT, QB * P], FP8)
            for u in range(DT):
                nc.vector.scalar_tensor_tensor(
                    out=xqa8[:, u, :], in0=xqT_sb[:, u, :], scalar=1.0,
                    in1=aq_b, op0=MUL, op1=MUL)
            wq8_sb = xap.tile([P, DT, D], FP8)
            nc.sync.dma_start(out=wq8_sb,
                              in_=wq8_d.rearrange("(u p) o -> p u o", p=P))
            csq_sb = xap.tile([32, 2, D], FP8)
            nc.sync.dma_start(out=csq_sb, in_=csq_d)
            for cidx in range(6):
                t, slot = divmod(cidx, 3)
                pmt = ps_pool.tile([P, 8 * P], F32, tag="ps8", name="pmt")
                pm = pmt[:, 0:QB * P]
                for g in range(3):
                    nc.tensor.matmul(pm, wq8_sb[:, 2 * g:2 * g + 2,
                                                cidx * P:(cidx + 1) * P],
                                     xqa8[:, 2 * g:2 * g + 2, :],
                                     start=(g == 0), stop=False, perf_mode=DRM)
                nc.tensor.matmul(pm, csq_sb[:, :, cidx * P:(cidx + 1) * P],
                                 bq_pad, start=False, stop=True, perf_mode=DRM,
                                 skip_group_check=True)
                nc.vector.tensor_copy(out=q8[:, t, slot, :], in_=pm)

            def build_units(sb):
                c0, c1 = sb * 512, (sb + 1) * 512
                us = []

                def xa_unit(u0):
                    def f():
                        for u in (u0, u0 + 1):
                            xab = xt_pool.tile([P, 512], BF16, tag="xab")
                            nc.vector.tensor_mul(out=xab, in0=xts[sb][:, u, :],
                                                 in1=a_b[:, c0:c1])
                            nc.gpsimd.tensor_copy(out=xa8[:, u, c0:c1],
                                                  in_=xab)
                    return f

                def k_unit(cidx):
                    def f():
                        t, slot = divmod(cidx, 3)
                        pm = pv_pool.tile([P, 512], F32, tag="pv1",
                                          name="pm")
                        for g in range(3):
                            nc.tensor.matmul(
                                pm, wk8_sb[:, 2 * g:2 * g + 2,
                                           cidx * P:(cidx + 1) * P],
                                xa8[:, 2 * g:2 * g + 2, c0:c1],
                                start=(g == 0), stop=False, perf_mode=DRM)
                        nc.tensor.matmul(
                            pm, csk_sb[:, :, cidx * P:(cidx + 1) * P],
                            b_pad[:, :, c0:c1], start=False, stop=True,
                            perf_mode=DRM, skip_group_check=True)
                        if cidx % 2 == 0:
                            nc.scalar.activation(out=k8[:, t, slot, c0:c1],
                                                 in_=pm, func=AF.Identity,
                                                 scale=1.0)
                        else:
                            nc.vector.tensor_copy(out=k8[:, t, slot, c0:c1],
                                                  in_=pm)
                    return f

                def v_unit(r):
                    def f():
                        blk = sb * 4 + r
                        bc0 = blk * P
                        pm1 = pv_pool.tile([P, 512], F32, tag="pv1")
                        pm2 = pv_pool.tile([P, 256], F32, tag="pv2")
                        for pm, cols in ((pm1, slice(0, 512)),
                                         (pm2, slice(512, 768))):
                            for g in range(3):
                                nc.tensor.matmul(
                                    pm,
                                    xa8[:, 2 * g:2 * g + 2, bc0:bc0 + P],
                                    wv8_sb[:, 2 * g:2 * g + 2, cols],
                                    start=(g == 0), stop=False, perf_mode=DRM)
                            nc.tensor.matmul(
                                pm, b_pad[:, :, bc0:bc0 + P],
                                csv_sb[:, :, cols], start=False, stop=True,
                                perf_mode=DRM, skip_group_check=True)
                        nc.vector.tensor_copy(
                            out=v8[:, blk, 0:8, 0:HD],
                            in_=pm1.rearrange("p (h d) -> p h d", h=8))
                        nc.vector.tensor_copy(
                            out=v8[:, blk, 8:12, 0:HD],
                            in_=pm2.rearrange("p (h d) -> p h d", h=4))
                    return f

                for u0 in range(0, DT, 2):
                    us.append(xa_unit(u0))
                for cidx in range(6):
                    us.append(k_unit(cidx))
                for r in range(4):
                    us.append(v_unit(r))
                return us

            def attn_i(i, units=()):
                nj = 8 * i + 8
                units = list(units)
                slots = 6 * (i + 1)
                nunits = len(units)
                slot_no = [0]

                def fill():
                    # Bresenham pacing: after slot k, ~(k+1)*n/slots units out
                    slot_no[0] += 1
                    target = (slot_no[0] * nunits + slots - 1) // slots
                    while units and nunits - len(units) < target:
                        units.pop(0)()

                def sc_part(h, g):
                    q0 = 32 * (h % 4)
                    hs = h // 4
                    ps8 = ps_pool.tile([P, 8 * P], F32, tag="ps8", name="ps8")
                    for jj in range(8):
                        j = 8 * g + jj
                        nc.tensor.matmul(
                            ps8[:, jj * P:(jj + 1) * P],
                            k8[q0:q0 + 32, :, hs, j * P:(j + 1) * P],
                            q8[q0:q0 + 32, :, hs, i * P:(i + 1) * P],
                            start=True, stop=True, perf_mode=DRM,
                            tile_position=(q0, 0))
                    return ps8

                def ea_part(h, g, ps8, first, last):
                    sl = 0
                    vals = vals_t[h]
                    if g == i:   # diagonal group: mask, bf16 AV
                        pT = att_pool.tile([P, 8 * P], BF16, tag="pTb")
                        nc.scalar.activation(out=pT, in_=ps8, func=AF.Exp,
                                             scale=ESC)
                        eng = nc.vector if (h % 2 == 0) else nc.gpsimd
                        eng.tensor_mul(
                            out=pT, in0=pT,
                            in1=killd_t[:, i * 1024:(i + 1) * 1024])
                        for jj in range(8):
                            nc.tensor.matmul(
                                vals[:, sl:sl + HD1],
                                pT[:, jj * P:(jj + 1) * P],
                                v8[:, 8 * g + jj, h, :],
                                start=(first and jj == 0),
                                stop=(last and jj == 7),
                                skip_group_check=True)
                    else:        # fully-live group: fp8 probs, DR AV
                        pT8 = att_pool.tile([P, 8 * P], FP8, tag="pT8")
                        nc.scalar.activation(out=pT8, in_=ps8,
                                             func=AF.Exp, scale=ESC)
                        p2 = pT8.rearrange("p (m q) -> p m q", m=8)
                        for m in range(4):
                            nc.tensor.matmul(
                                vals[:, sl:sl + HD1],
                                p2[:, 2 * m:2 * m + 2, :],
                                v8[:, 8 * g + 2 * m:8 * g + 2 * m + 2, h, :],
                                start=(first and m == 0),
                                stop=(last and m == 3), perf_mode=DRM,
                                skip_group_check=True)

                def drain_part(h):
                    sl = 0
                    vals = vals_t[h]
                    rs = att_pool.tile([P, 1], F32, tag="rs")
                    nc.vector.reciprocal(out=rs, in_=vals[:, sl + HD:sl + HD1])
                    nc.vector.tensor_scalar_mul(
                        out=attn_sb[:, i, h * HD:(h + 1) * HD],
                        in0=vals[:, sl:sl + HD], scalar1=rs)

                vals_t = {}
                gorder = [i] + list(range(i))   # diagonal group first
                for hp in range(0, H, 2):
                    vals_t[hp] = pb_pool.tile([P, 512], F32, tag="vals",
                                              name="vals")
                    vals_t[hp + 1] = pb_pool.tile([P, 512], F32, tag="vals",
                                                  name="vals")
                    h0, h1 = hp, hp + 1
                    for n, g in enumerate(gorder):
                        psa = sc_part(h0, g)
                        psb = sc_part(h1, g)
                        fill()
                        ea_part(h0, g, psa, n == 0, n == i)
                        ea_part(h1, g, psb, n == 0, n == i)
                    drain_part(h0)
                    drain_part(h1)
                while units:
                    units.pop(0)()

            def wo_part(i):
                """Wo + x1T for query block i (pipelined after attn_i(i))."""
                nc.sync.dma_start_transpose(out=attn_T[:, :, i, :],
                                            in_=attn_sb[:, i, :])
                pw1 = pv_pool.tile([P, 512], F32, tag="pv1")
                pw2 = pv_pool.tile([P, 256], F32, tag="pv2")
                for ot in range(6):
                    pm = pw1[:, ot * P:(ot + 1) * P] if ot < 4 else \
                        pw2[:, (ot - 4) * P:(ot - 3) * P]
                    for dt in range(DT):
                        nc.tensor.matmul(
                            pm,
                            wo_sb[:, dt, ot * P:(ot + 1) * P],
                            attn_T[:, dt, i, :], start=(dt == 0), stop=False,
                            skip_group_check=True)
                    nc.tensor.matmul(pm,
                                     cso_sb[:, :, ot * P:(ot + 1) * P],
                                     bq_pad[:, :, i * P:(i + 1) * P],
                                     start=False, stop=True, perf_mode=DRM,
                                     skip_group_check=True)
                nc.vector.tensor_add(
                    out=x1T[:, 0:4, i * P:(i + 1) * P],
                    in0=pw1.rearrange("p (u c) -> p u c", u=4),
                    in1=xqT_sb[:, 0:4, i * P:(i + 1) * P])
                nc.vector.tensor_add(
                    out=x1T[:, 4:6, i * P:(i + 1) * P],
                    in0=pw2.rearrange("p (u c) -> p u c", u=2),
                    in1=xqT_sb[:, 4:6, i * P:(i + 1) * P])
                x1r = st_pool.tile([P, DT, P], BF16, tag="x1r")
                for ot in range(6):
                    nc.sync.dma_start_transpose(
                        out=x1r[:, ot, :], in_=x1T[:, ot, i * P:(i + 1) * P])
                bn_block(st_pool, x1r.rearrange("p u c -> p (u c)"),
                         mv_blk[:, 36 + i, :])

            # ---- stats + build + attention pipeline: builds/stats of
            # later rounds are interleaved INTO the attention emission so no
            # engine queue ever parks behind another engine's work ----
            xts = {}

            def stats_units(half):
                us = []

                def sb_unit(sb):
                    def f():
                        xs = xs_pool.tile([P, 4, D], BF16, tag="xs")
                        nc.sync.dma_start(
                            out=xs,
                            in_=xrow_d[sb * 512:(sb + 1) * 512, :].rearrange(
                                "(r p) f -> p r f", p=P))
                        for r in range(4):
                            bn_block(st_pool, xs[:, r, :],
                                     mv_blk[:, sb * 4 + r, :])
                        xt = xt_pool.tile([P, DT, 512], BF16, tag="xt")
                        nc.sync.dma_start(
                            out=xt,
                            in_=xT_d.rearrange("(u p) s -> p u s", p=P)[
                                :, :, sb * 512:(sb + 1) * 512])
                        xts[sb] = xt
                    return f

                for sb in (2 * half, 2 * half + 1):
                    us.append(sb_unit(sb))
                return us

            def rstd_rows(r0, r1):
                n = r1 - r0
                rstd_batch(st_pool, mv_blk[:, r0:r1, :],
                           a_blk[:, r0:r1], b_blk[:, r0:r1])
                bounce(st_pool, a_blk, b_blk, scra_d, scrb_d, r0, r1, d0=r0)
                nc.sync.dma_start(
                    out=a_b[:, r0 * P:r1 * P],
                    in_=scra_d[:, r0 * P:r1 * P].to_broadcast([P, n * P]))
                nc.gpsimd.dma_start(out=b_pad[0:1, 0, r0 * P:r1 * P],
                                    in_=scrb_d[:, r0 * P:r1 * P])

            def rstd_bcast(half):
                rstd_rows(half * 8, half * 8 + 8)

            su0 = stats_units(0)
            su0[0]()                 # sb0 stats
            rstd_rows(0, 4)
            b0 = build_units(0)
            for u in b0[0:3]:        # xa8(sb0)
                u()
            su0[1]()                 # sb1 stats
            rstd_rows(4, 8)
            for u in b0[3:] + build_units(1):
                u()
            for u in stats_units(1):
                u()
            rstd_bcast(1)

            nc.sync.dma_start(out=killd_t, in_=killd_d)
            nc.sync.dma_start(out=wo_sb,
                              in_=wo_d.rearrange("(u p) o -> p u o", p=P))
            nc.sync.dma_start(out=cso_sb, in_=cso_d)
            attn_i(0, build_units(2) + build_units(3) + stats_units(2))
            wo_part(0)
            rstd_bcast(2)
            attn_i(1, build_units(4) + build_units(5) + stats_units(3))
            wo_part(1)
            rstd_bcast(3)
            attn_i(2, build_units(6) + build_units(7))
            wo_part(2)

            xa_scope.close()

            # FFN1 weights load during the i=3 attention tail
            fw = s1.enter_context(tc.tile_pool(name="fw", bufs=1))
            fw1_scope = ExitStack()
            fw1 = fw1_scope.enter_context(tc.tile_pool(name="fw1", bufs=1))
            w18_sb = fw1.tile([P, DT, HID], FP8)
            nc.sync.dma_start(out=w18_sb,
                              in_=w18_d.rearrange("(u p) o -> p u o", p=P))
            if USE_LO1:
                w18lo_sb = fw1.tile([P, DT, HID], FP8)
                nc.sync.dma_start(
                    out=w18lo_sb,
                    in_=w18lo_d.rearrange("(u p) o -> p u o", p=P))
            csw1_sb = fw1.tile([32, 2, HID], FP8)
            nc.sync.dma_start(out=csw1_sb, in_=csw1_d)

            attn_i(3)
            wo_part(3)

            # ---- LN2 rstd + x1a8 ----
            rstd_batch(st_pool, mv_blk[:, 36:40, :], a_blk[:, 36:40],
                       b_blk[:, 36:40])
            bounce(st_pool, a_blk, b_blk, scra2_d, scrb2_d, 36, 40, d0=0)
            a2_b = fw.tile([P, QB * P], BF16)
            nc.sync.dma_start(out=a2_b, in_=scra2_d.to_broadcast([P, QB * P]))
            b2_pad = fw.tile([32, 2, QB * P], FP8)
            nc.sync.dma_start(out=b2_pad, in_=bones_d[:, :, 0:QB * P])
            nc.gpsimd.dma_start(out=b2_pad[0:1, 0, :], in_=scrb2_d)
            x1a8 = fw.tile([P, DT, QB * P], FP8)
            for u in range(DT):
                nc.vector.scalar_tensor_tensor(
                    out=x1a8[:, u, :], in0=x1T[:, u, :], scalar=1.0,
                    in1=a2_b, op0=MUL, op1=MUL)

            # ---- FFN ----
            h8 = fw.tile([P, 2, FT // 2, QB * P], FP8)
            for ft in range(FT):
                pmt = ps_pool.tile([P, 8 * P], F32, tag="ps8", name="pmt")
                pm = pmt[:, 0:QB * P]
                for g in range(3):
                    nc.tensor.matmul(pm, w18_sb[:, 2 * g:2 * g + 2,
                                                ft * P:(ft + 1) * P],
                                     x1a8[:, 2 * g:2 * g + 2, :],
                                     start=(g == 0), stop=False, perf_mode=DRM)
                if USE_LO1:
                    for g in range(3):
                        nc.tensor.matmul(pm, w18lo_sb[:, 2 * g:2 * g + 2,
                                                      ft * P:(ft + 1) * P],
                                         x1a8[:, 2 * g:2 * g + 2, :],
                                         start=False, stop=False,
                                         perf_mode=DRM, skip_group_check=True)
                nc.tensor.matmul(pm, csw1_sb[:, :, ft * P:(ft + 1) * P],
                                 b2_pad, start=False, stop=True,
                                 perf_mode=DRM, skip_group_check=True)
                nc.scalar.activation(out=h8[:, ft % 2, ft // 2, :], in_=pm,
                                     func=AF.Gelu, scale=1.0 / KS)
            fw1_scope.close()
            fw2 = s1.enter_context(tc.tile_pool(name="fw2", bufs=1))
            csw2_sb = fw2.tile([32, 2, D], FP8)
            nc.sync.dma_start(out=csw2_sb, in_=csw2_d)
            w28_sb = fw2.tile([P, 2, FT // 2, D], FP8)
            nc.sync.dma_start(
                out=w28_sb.rearrange("p q v o -> p (q v) o"),
                in_=w28_d.rearrange("(u p) o -> p u o", p=P))
            if USE_LO2:
                w28lo_sb = fw2.tile([P, 2, FT // 2, D], FP8)
                nc.sync.dma_start(
                    out=w28lo_sb.rearrange("p q v o -> p (q v) o"),
                    in_=w28lo_d.rearrange("(u p) o -> p u o", p=P))
            outT = fw.tile([P, DT, QB * P], BF16)
            for ot in range(6):
                pmt = ps_pool.tile([P, 8 * P], F32, tag="ps8", name="pmt")
                pm = pmt[:, 0:QB * P]
                for v in range(FT // 2):
                    nc.tensor.matmul(pm, w28_sb[:, :, v, ot * P:(ot + 1) * P],
                                     h8[:, :, v, :], start=(v == 0),
                                     stop=False, perf_mode=DRM)
                if USE_LO2:
                    for v in range(FT // 2):
                        nc.tensor.matmul(pm,
                                         w28lo_sb[:, :, v,
                                                  ot * P:(ot + 1) * P],
                                         h8[:, :, v, :], start=False,
                                         stop=False, perf_mode=DRM,
                                         skip_group_check=True)
                nc.tensor.matmul(pm, csw2_sb[:, :, ot * P:(ot + 1) * P],
                                 bq_pad, start=False, stop=True,
                                 perf_mode=DRM, skip_group_check=True)
                nc.vector.scalar_tensor_tensor(
                    out=outT[:, ot, :], in0=pm, scalar=1.0 / WS,
                    in1=x1T[:, ot, :], op0=MUL, op1=ADD)
                o_r = st_pool.tile([P, QB, P], BF16, tag="or")
                nc.sync.dma_start_transpose(out=o_r, in_=outT[:, ot, :])
                nc.sync.dma_start(
                    out=out_d.rearrange("(r p) f -> p r f", p=P)[
                        :, :, ot * P:(ot + 1) * P],
                    in_=o_r)

    nc.compile()
    return nc


def _kq_perm():
    """Column permutation for Wk/Wq: tile cidx = t*3 + slot, col j ->
    original feature 64*h + d with h = slot*4 + j//32, d = t*32 + j%32."""
    perm = np.zeros(D, np.int64)
    for cidx in range(6):
        t, slot = divmod(cidx, 3)
        for j in range(P):
            h = slot * 4 + j // 32
            d = t * 32 + (j % 32)
            perm[cidx * P + j] = 64 * h + d
    return perm


def _f8(x):
    return np.clip(np.asarray(x, np.float64), -240.0, 240.0).astype(f8dt)


def _cs_pack(cs_row, bias_row, n):
    """[32, 2, n] fp8 rank-1 lhsT: (p=0,t=0) = cs_row, (p=1,t=1) = bias_row."""
    m = np.zeros((32, 2, n), np.float64)
    m[0, 0, :] = cs_row
    m[1, 1, :] = bias_row
    return _f8(m)


def _prep_inputs(x, gamma1, beta1, Wqkv, bqkv, Wo, bo, gamma2, beta2,
                 W1, b1, W2, b2):
    x2 = np.asarray(x, np.float64).reshape(S, D)
    g1 = np.asarray(gamma1, np.float64)
    be1 = np.asarray(beta1, np.float64)
    g2 = np.asarray(gamma2, np.float64)
    be2 = np.asarray(beta2, np.float64)
    W4 = np.asarray(Wqkv, np.float64).reshape(D, H, 3, HD)
    b4 = np.asarray(bqkv, np.float64).reshape(H, 3, HD)
    wq = W4[:, :, 0, :].reshape(D, D)
    wk = W4[:, :, 1, :].reshape(D, D)
    wv = W4[:, :, 2, :].reshape(D, D)
    bq = b4[:, 0, :].reshape(D)
    bk = b4[:, 1, :].reshape(D)
    bv = b4[:, 2, :].reshape(D)
    Wo = np.asarray(Wo, np.float64)
    bo = np.asarray(bo, np.float64)
    W1 = np.asarray(W1, np.float64)
    b1 = np.asarray(b1, np.float64)
    W2 = np.asarray(W2, np.float64)
    b2 = np.asarray(b2, np.float64)

    def _w2p(W):
        return WS * W.reshape(FT // 2, 2, P, D).transpose(1, 0, 2, 3).reshape(
            HID, D)

    perm = _kq_perm()
    wq_t = g1[:, None] * wq
    wk_t = g1[:, None] * wk
    wv_t = g1[:, None] * wv
    bq_t = bq + be1 @ wq
    bk_t = bk + be1 @ wk
    bv_t = bv + be1 @ wv
    w1_t = g2[:, None] * W1
    b1_t = b1 + be2 @ W1

    bones = np.zeros((32, 2, S), f8dt)
    bones[1, 1, :] = f8dt(1.0)

    common = {
        "xrow": x2.astype(bfdt),
        "xT": np.ascontiguousarray(x2.T).astype(bfdt),
        "wk8": _f8(WS * wk_t[:, perm]),
        "wq8": _f8(WS * wq_t[:, perm]),
        "wv8": _f8(WS * wv_t),
        "wo": np.ascontiguousarray(Wo / KS).astype(bfdt),
        "w18": _f8(WS * w1_t),
        "w18lo": _f8(WS * w1_t - _f8(WS * w1_t).astype(np.float64)),
        "w28": _f8(_w2p(W2)),
        "w28lo": _f8(_w2p(W2) - _f8(_w2p(W2)).astype(np.float64)),
        "csk": _cs_pack(WS * wk_t.sum(0)[perm], KS * bk_t[perm], D),
        "csq": _cs_pack(WS * wq_t.sum(0)[perm], KS * bq_t[perm], D),
        "csv": _cs_pack(WS * wv_t.sum(0), KS * bv_t, D),
        "csw1": _cs_pack(WS * w1_t.sum(0), KS * b1_t, HID),
        "csw2": _cs_pack(np.zeros(D), WS * b2, D),
        "cso": _cs_pack(np.zeros(D), bo, D),
        "bones": bones,
    }

    xb = x2.reshape(NB, P, D)
    tri_T = np.tril(np.ones((P, P), np.float64)).T  # [k, q] k<=q
    in_maps = []
    for c in range(NCORES):
        blocks = [c + 8 * i for i in range(QB)]
        xq = np.ascontiguousarray(xb[blocks].reshape(QB * P, D))
        killd = np.zeros((P, QB * 8 * P), np.float64)
        for i in range(QB):
            for jj in range(8):
                t0 = (i * 8 + jj) * P
                if jj < c:
                    killd[:, t0:t0 + P] = 1.0
                elif jj == c:
                    killd[:, t0:t0 + P] = tri_T
        m = dict(common)
        m["xq"] = xq.astype(bfdt)
        m["xqT"] = np.ascontiguousarray(xq.T).astype(bfdt)
        m["killd"] = killd.astype(bfdt)
        in_maps.append(m)
    return in_maps


def kernel(**inputs):
    nc = _CACHE.get("nc")
    if nc is None:
        nc = _build()
        _CACHE["nc"] = nc
    in_maps = _prep_inputs(**inputs)
    res = run_bass_kernel_spmd(nc, in_maps, list(range(NCORES)))
    out = np.zeros((S, D), np.float32)
    ob = out.reshape(NB, P, D)
    for c in range(NCORES):
        o = np.asarray(res.results[c]["out"], dtype=np.float32).reshape(
            QB, P, D)
        for i in range(QB):
            ob[c + 8 * i] = o[i]
    return out.reshape(1, S, D)

